# revision 23
# baseline (speedup 1.0000x reference)
"""Trainium2 Bass kernel: 12-head self-attention (B=8, N=1024, D=768).

Sharding: data-parallel over batch - one batch element per NeuronCore,
weights replicated on all 8 cores, no collectives.

Per-core dataflow (all matmuls bf16 operands, fp32 PSUM accumulation):
  xT [768,1024] (host-pretransposed, bf16, chunk-major DRAM layout)
  qkT[t] = W_qk[:,t-chunk].T @ xT          (feature-major q/k, 12 tiles)
  v[mt]  = xT[:,mt-chunk].T @ W_v          (token-major v; per-head slot
                                            [ones | 64 v cols], VW=65)
  per head h:
    S^T[mt] = kT_h[:,mt].T @ qT_h          ([keys,queries], K=64)
    P^T[mt] = exp(scale * S^T[mt])         (ACT; scores ~N(0,1), safe)
    outT   += v'_h[mt].T @ P^T[mt]         (sums -> PSUM row 0, data rows
                                            1..64)
    attn_T_h = outT[1:65] * bcast(1/outT[0])  (-> attn_sb via DMA hop)
  outT[ft] = W_p[:,ft-cols].T @ attn_T     (proj transposed: feature-major
                                            output, per-partition bias via
                                            tensor_scalar; host transposes)

Startup: inputs live in DRAM pre-packed to match SBUF layout (one dense
descriptor each); xT chunks split across sync+vector queues, first-needed
W_qk tiles on scalar, the rest behind W_v on gpsimd. The first two qkT
tiles are emitted chunk-major so the PE starts on xT chunk 0 arrival.

Steady state: head-sequential pipeline, 3 rotating psA slots + 1 psB PV
accumulator; PV lags ST/exp by one mt step; filler qkT/v matmuls are
compressed into the first ~6 mt steps of each head so the parked filler
PSUM slot frees before the next head's STs need it.

Tail: proj runs transposed (rhs = attn chunks, 1024-col streams); first
chunks of ft tiles 0-2 are emitted inside the last head's drain so they
overlap the final exp/norm chain.
"""

from contextlib import ExitStack

import numpy as np
import ml_dtypes

import concourse.bacc as bacc
import concourse.bass as bass
import concourse.mybir as mybir
import concourse.tile as tile
from concourse.bass_utils import run_bass_kernel_spmd

B, N, D = 8, 1024, 768
H, HD = 12, 64
SCALE = HD ** -0.5
KC = D // 128          # 6 contraction chunks of 128
NT = N // 128          # 8 token tiles of 128
FT = D // 128          # 6 output-feature tiles of 128 (proj)
VW = 65                # per-head v slot: col 0 = ones, cols 1..64 = v data
F32 = mybir.dt.float32
BF16 = mybir.dt.bfloat16
NCORES = 8

_CACHE = {}


def _build_nc():
    nc = bacc.Bacc(None, target_bir_lowering=False)
    # DRAM layouts pre-packed on host to match SBUF destination exactly.
    xT = nc.dram_tensor("xT", [128, KC, N], BF16, kind="ExternalInput")
    w_qk = nc.dram_tensor("w_qk", [128, 2 * KC, KC, 128], BF16, kind="ExternalInput")
    w_v = nc.dram_tensor("w_v", [128, KC, D], BF16, kind="ExternalInput")
    w_p = nc.dram_tensor("w_p", [128, KC, D], BF16, kind="ExternalInput")
    bias = nc.dram_tensor("bias", [128, FT], F32, kind="ExternalInput")
    outT = nc.dram_tensor("outT", [D, N], F32, kind="ExternalOutput")

    with ExitStack() as ctx:
        tc = ctx.enter_context(tile.TileContext(nc))
        const = ctx.enter_context(tc.tile_pool(name="const", bufs=1))
        work = ctx.enter_context(tc.tile_pool(name="work", bufs=2))
        # PSUM: 8 banks. psA = 3 rotating [128,1024] f32 slots (6 banks),
        # psB = 1 slot (2 banks, PV accumulator / 4th proj tile).
        psA = ctx.enter_context(tc.tile_pool(name="psA", bufs=3, space="PSUM"))
        psB = ctx.enter_context(tc.tile_pool(name="psB", bufs=1, space="PSUM"))

        xT_sb = const.tile([128, KC, N], BF16)
        wqk_sb = const.tile([128, 2 * KC, KC, 128], BF16)
        wv_sb = const.tile([128, KC, D], BF16)
        wp_sb = const.tile([128, KC, D], BF16)
        bias_sb = const.tile([128, FT], F32)
        qk_sb = const.tile([128, 2 * KC, N], BF16)   # tiles 0-5: qT, 6-11: kT
        v_sb = const.tile([128, NT, H, VW], BF16)
        attn_sb = const.tile([128, KC, N], BF16)     # attn_out^T, normalized

        # --- DMA schedule (priority order per queue; only sync/scalar
        # HWDGE + gpsimd SWDGE can trigger DMAs) ---
        # xT chunks split across sync + scalar so the tensor the whole
        # startup is gated on arrives ~2x faster; first-needed W_qk tiles
        # (q0, k0, then q1, k1) interleaved on scalar.
        for c in (0, 1, 2, 3):
            nc.sync.dma_start(out=xT_sb[:, c, :], in_=xT[:, c, :])
        nc.scalar.dma_start(out=wqk_sb[:, 0], in_=w_qk[:, 0])
        nc.scalar.dma_start(out=wqk_sb[:, KC], in_=w_qk[:, KC])
        for c in (4, 5):
            nc.scalar.dma_start(out=xT_sb[:, c, :], in_=xT[:, c, :])
        for t in (1, KC + 1):
            nc.scalar.dma_start(out=wqk_sb[:, t], in_=w_qk[:, t])
        # Late weights ride the scalar ring's FIFO behind the startup-
        # critical loads, so they can't steal DMA bandwidth from xT/W_v
        # during the ramp. gpsimd (SWDGE) carries only W_v.
        nc.scalar.dma_start(out=wqk_sb[:, 2:KC], in_=w_qk[:, 2:KC])
        nc.scalar.dma_start(out=wqk_sb[:, KC + 2:], in_=w_qk[:, KC + 2:])
        nc.scalar.dma_start(out=wp_sb, in_=w_p[:, :, :])
        nc.scalar.dma_start(out=bias_sb, in_=bias[:, :])
        nc.gpsimd.dma_start(out=wv_sb, in_=w_v[:, :, :])
        # v' scaffold: only the ones column needs initialising (cols 1..64
        # are fully written by the v casts).
        nc.vector.memset(v_sb[:, :, :, 0:1], 1.0)

        def qkT_ops(t):
            """Closures: 6 accumulation-chunk matmul pairs + the cast copy,
            for interleaving as PE filler inside a head's mt loop."""
            ps_qk = psA.tile([128, N], F32, tag="ps", name="ps_qk")
            ops = []
            for c in range(KC):
                def chunk(c=c, ps_qk=ps_qk):
                    for s in range(2):
                        nc.tensor.matmul(
                            ps_qk[:, 512 * s:512 * (s + 1)],
                            lhsT=wqk_sb[:, t, c, :],
                            rhs=xT_sb[:, c, 512 * s:512 * (s + 1)],
                            start=(c == 0), stop=(c == KC - 1),
                        )
                ops.append(chunk)

            def fin(ps_qk=ps_qk):
                nc.vector.tensor_copy(out=qk_sb[:, t, :], in_=ps_qk)
            ops.append(fin)
            return ops

        def emit_qkT_startup():
            # q-tile 0 and k-tile 0 emitted chunk-major so the PE starts as
            # soon as xT chunk 0 + the first W_qk tile arrive.
            ps0 = psA.tile([128, N], F32, tag="ps", name="ps_qk0")
            ps1 = psA.tile([128, N], F32, tag="ps", name="ps_qk1")
            for c in range(KC):
                for t, ps in ((0, ps0), (KC, ps1)):
                    for s in range(2):
                        nc.tensor.matmul(
                            ps[:, 512 * s:512 * (s + 1)],
                            lhsT=wqk_sb[:, t, c, :],
                            rhs=xT_sb[:, c, 512 * s:512 * (s + 1)],
                            start=(c == 0), stop=(c == KC - 1),
                        )
            nc.vector.tensor_copy(out=qk_sb[:, 0, :], in_=ps0)
            nc.vector.tensor_copy(out=qk_sb[:, KC, :], in_=ps1)

        def emit_v(mt):
            ps_v = psA.tile([128, N], F32, tag="ps", name="ps_v")
            for c in range(KC):
                for lo, sz in ((0, 512), (512, 256)):
                    nc.tensor.matmul(
                        ps_v[:, lo:lo + sz],
                        lhsT=xT_sb[:, c, 128 * mt:128 * (mt + 1)],
                        rhs=wv_sb[:, c, lo:lo + sz],
                        start=(c == 0), stop=(c == KC - 1),
                    )
            nc.vector.tensor_copy(
                out=v_sb[:, mt, :, 1:VW],
                in_=ps_v[:, 0:D].rearrange("p (h e) -> p h e", e=HD),
            )

        ps_o_map = {}
        norm_pending = {}

        def emit_ST_exp(h, mt):
            tq, tk = h // 2, KC + h // 2
            po = (h % 2) * 64
            ps_s = psA.tile([128, N], F32, tag="ps", name="ps_s")
            for s in range(2):
                nc.tensor.matmul(
                    ps_s[:, 512 * s:512 * (s + 1)],
                    lhsT=qk_sb[po:po + 64, tk, 128 * mt:128 * (mt + 1)],
                    rhs=qk_sb[po:po + 64, tq, 512 * s:512 * (s + 1)],
                    start=True, stop=True,
                )
            pt = work.tile([128, N], BF16, tag="pt", name="pt", bufs=8)
            nc.scalar.activation(
                out=pt, in_=ps_s,
                func=mybir.ActivationFunctionType.Exp, scale=SCALE,
            )
            return pt

        def emit_PV(h, mt, pt):
            if mt == 0:
                ps_o_map[h] = psB.tile([128, N], F32, tag="ps", name="ps_o")
            ps_o = ps_o_map[h]
            for s in range(2):
                nc.tensor.matmul(
                    ps_o[0:VW, 512 * s:512 * (s + 1)],
                    lhsT=v_sb[:, mt, h, :],
                    rhs=pt[:, 512 * s:512 * (s + 1)],
                    start=(mt == 0), stop=(mt == NT - 1),
                )

        def emit_norm_a(h):
            # sums on PSUM partition 0 (v' col 0 = ones); v data on
            # partitions 64..127. One bf16 cast moves everything off PSUM
            # so the psB slot frees ~3us earlier (the next head's first PV
            # aliases it); recip/broadcast/mul then work from the copy.
            # reciprocal_approx_fast / partition_broadcast read base
            # partition 0 on HW; DVE can't shift partitions, so even heads
            # hop via SBUF->SBUF DMA.
            ps_o = ps_o_map.pop(h)
            u = work.tile([128, N], F32, tag="u", name="u")
            nc.vector.tensor_copy(out=u[0:VW, :], in_=ps_o[0:VW, :])
            recip = work.tile([1, N], F32, tag="recip", name="recip")
            nc.vector.reciprocal_approx_fast(out=recip, in_=u[0:1, :])
            rb = work.tile([128, N], F32, tag="rb", name="rb")
            nc.gpsimd.partition_broadcast(rb, recip)
            norm_pending[h] = (u, rb)

        def emit_norm_b(h):
            # mul over partitions 0..64 (base 0 keeps DVE alignment; the
            # partition-0 product is sums/sums, never read), then every
            # head hops rows 1..64 into its attn_sb slot via DMA.
            tq = h // 2
            po = (h % 2) * 64
            u, rb = norm_pending.pop(h)
            tmp = work.tile([128, N], BF16, tag="tmp", name="tmp")
            nc.vector.tensor_mul(
                out=tmp[0:VW, :], in0=u[0:VW, :], in1=rb[0:VW, :],
            )
            nc.sync.dma_start(
                out=attn_sb[po:po + 64, tq, :], in_=tmp[1:VW, :],
            )

        # Transposed proj: out^T[ft] = sum_c W_p[c-rows, ft-cols].T @ attn_c
        proj_ps = {}

        def emit_proj_chunks(ft, cs, pool):
            if ft not in proj_ps:
                proj_ps[ft] = pool.tile([128, N], F32, tag="ps", name="ps_p")
            ps_p = proj_ps[ft]
            for c in cs:
                for s in range(2):
                    nc.tensor.matmul(
                        ps_p[:, 512 * s:512 * (s + 1)],
                        lhsT=wp_sb[:, c, 128 * ft:128 * (ft + 1)],
                        rhs=attn_sb[:, c, 512 * s:512 * (s + 1)],
                        start=(c == 0), stop=(c == KC - 1),
                    )

        def emit_proj_fin(ft):
            # halves pipeline: the bias-add + out-DMA of cols 0-511 overlap
            # the second half's matmul drain
            ps_p = proj_ps.pop(ft)
            o_sb = work.tile([128, N], F32, tag="o_sb", name="o_sb", bufs=6)
            dq = nc.sync if ft % 2 == 0 else nc.scalar
            for s in range(2):
                sl = slice(512 * s, 512 * (s + 1))
                nc.vector.tensor_scalar_add(
                    out=o_sb[:, sl], in0=ps_p[:, sl],
                    scalar1=bias_sb[:, ft:ft + 1],
                )
                dq.dma_start(out=outT[128 * ft:128 * (ft + 1), sl], in_=o_sb[:, sl])

        # --- schedule ---
        emit_qkT_startup()
        for mt in range(2):
            emit_v(mt)
        # Filler PE work interleaved inside each head's mt loop, compressed
        # into the first ~6 steps so the parked filler PSUM slot (and its
        # DVE cast) complete before the next head's STs rotate into it.
        head_fillers = {h: [] for h in range(H)}
        head_fillers[0] = [
            (lambda mt=mt: emit_v(mt)) for mt in range(2, NT)
        ] + qkT_ops(1)
        head_fillers[1] = qkT_ops(KC + 1)
        for k in range(1, KC - 1):
            head_fillers[2 * k] = qkT_ops(k + 1)
            head_fillers[2 * k + 1] = qkT_ops(KC + k + 1)

        # Software pipeline: PV lags ST/exp by one mt step.
        LAG = 1
        pending = []
        for h in range(H):
            fl = head_fillers[h]
            fi = 0
            for mt in range(NT):
                pt = emit_ST_exp(h, mt)
                pending.append((h, mt, pt))
                if len(pending) > LAG:
                    ph, pmt, ppt = pending.pop(0)
                    emit_PV(ph, pmt, ppt)
                    if pmt == NT - 1:
                        emit_norm_a(ph)
                    elif pmt == 3 and ph > 0:
                        emit_norm_b(ph - 1)
                while fi < min((mt + 1) * len(fl) // 6, len(fl)):
                    fl[fi]()
                    fi += 1
        # Drain + tail: overlap proj with the last exp/norm chain. The
        # last PVs need the last exps, so a little proj work in front is
        # free PE time.
        emit_proj_chunks(0, range(0, 2), psA)
        (ph, pmt, ppt) = pending.pop(0)
        emit_PV(ph, pmt, ppt)
        emit_norm_a(ph)
        emit_norm_b(ph)
        emit_proj_chunks(0, range(2, 5), psA)
        emit_proj_chunks(1, range(0, 5), psA)
        emit_proj_chunks(2, range(0, 5), psA)
        emit_proj_chunks(3, range(0, 5), psB)   # slot frees after u-cast(11)
        for ft in range(4):
            emit_proj_chunks(ft, range(5, KC), None)
            emit_proj_fin(ft)
        for ft in range(4, KC):
            emit_proj_chunks(ft, range(KC), psA)
            emit_proj_fin(ft)

    nc.compile()
    return nc


def _get_nc():
    if "nc" not in _CACHE:
        _CACHE["nc"] = _build_nc()
    return _CACHE["nc"]


def _make_in_maps(x, W_qkv, W_proj, b_proj):
    bf = ml_dtypes.bfloat16
    x = np.asarray(x, dtype=np.float32)
    W_qkv = np.asarray(W_qkv, dtype=np.float32)
    W_proj = np.asarray(W_proj, dtype=np.float32)
    b_proj = np.asarray(b_proj, dtype=np.float32)
    # Pre-pack DRAM layouts to match SBUF destinations (chunk-major per
    # partition) so each tensor is one dense DMA.
    w_qk = np.ascontiguousarray(
        W_qkv[:, :2 * D].reshape(KC, 128, 2 * KC, 128).transpose(1, 2, 0, 3)
    ).astype(bf)
    w_v = np.ascontiguousarray(
        W_qkv[:, 2 * D:].reshape(KC, 128, D).transpose(1, 0, 2)
    ).astype(bf)
    w_p = np.ascontiguousarray(
        W_proj.reshape(KC, 128, D).transpose(1, 0, 2)
    ).astype(bf)
    bias = np.ascontiguousarray(b_proj.reshape(FT, 128).T)
    xTs = [
        np.ascontiguousarray(
            x[b].T.reshape(KC, 128, N).transpose(1, 0, 2)
        ).astype(bf)
        for b in range(NCORES)
    ]
    return [
        {
            "xT": xTs[b],
            "w_qk": w_qk,
            "w_v": w_v,
            "w_p": w_p,
            "bias": bias,
        }
        for b in range(NCORES)
    ]


def run(x, W_qkv, W_proj, b_proj, trace=False):
    nc = _get_nc()
    in_maps = _make_in_maps(x, W_qkv, W_proj, b_proj)
    res = run_bass_kernel_spmd(nc, in_maps, core_ids=list(range(NCORES)), trace=trace)
    out = np.stack(
        [res.results[b]["outT"].T for b in range(NCORES)], axis=0
    )
    return np.ascontiguousarray(out).astype(np.float32), res


def kernel(x, W_qkv, W_proj, b_proj):
    out, _ = run(x, W_qkv, W_proj, b_proj, trace=False)
    return out


# revision 24
# speedup vs baseline: 1.0060x; 1.0060x over previous
"""Trainium2 Bass kernel: 12-head self-attention (B=8, N=1024, D=768).

Sharding: data-parallel over batch - one batch element per NeuronCore,
weights replicated on all 8 cores, no collectives.

Per-core dataflow (all matmuls bf16 operands, fp32 PSUM accumulation):
  xT [768,1024] (host-pretransposed, bf16, chunk-major DRAM layout)
  qkT[t] = W_qk[:,t-chunk].T @ xT          (feature-major q/k, 12 tiles)
  v[mt]  = xT[:,mt-chunk].T @ W_v          (token-major v; per-head slot
                                            [ones | 64 v cols], VW=65)
  per head h:
    S^T[mt] = kT_h[:,mt].T @ qT_h          ([keys,queries], K=64)
    P^T[mt] = exp(scale * S^T[mt])         (ACT; scores ~N(0,1), safe)
    outT   += v'_h[mt].T @ P^T[mt]         (sums -> PSUM row 0, data rows
                                            1..64)
    attn_T_h = outT[1:65] * bcast(1/outT[0])  (-> attn_sb via DMA hop)
  outT[ft] = W_p[:,ft-cols].T @ attn_T     (proj transposed: feature-major
                                            output, per-partition bias via
                                            tensor_scalar; host transposes)

Startup: inputs live in DRAM pre-packed to match SBUF layout (one dense
descriptor each); xT chunks split across sync+vector queues, first-needed
W_qk tiles on scalar, the rest behind W_v on gpsimd. The first two qkT
tiles are emitted chunk-major so the PE starts on xT chunk 0 arrival.

Steady state: head-sequential pipeline, 3 rotating psA slots + 1 psB PV
accumulator; PV lags ST/exp by one mt step; filler qkT/v matmuls are
compressed into the first ~6 mt steps of each head so the parked filler
PSUM slot frees before the next head's STs need it.

Tail: proj runs transposed (rhs = attn chunks, 1024-col streams); first
chunks of ft tiles 0-2 are emitted inside the last head's drain so they
overlap the final exp/norm chain.
"""

from contextlib import ExitStack

import numpy as np
import ml_dtypes

import concourse.bacc as bacc
import concourse.bass as bass
import concourse.mybir as mybir
import concourse.tile as tile
from concourse.bass_utils import run_bass_kernel_spmd

B, N, D = 8, 1024, 768
H, HD = 12, 64
SCALE = HD ** -0.5
KC = D // 128          # 6 contraction chunks of 128
NT = N // 128          # 8 token tiles of 128
FT = D // 128          # 6 output-feature tiles of 128 (proj)
VW = 128               # per-head v slot: col 0 = ones, cols 64..127 = v data
F32 = mybir.dt.float32
BF16 = mybir.dt.bfloat16
NCORES = 8

_CACHE = {}


def _build_nc():
    nc = bacc.Bacc(None, target_bir_lowering=False)
    # DRAM layouts pre-packed on host to match SBUF destination exactly.
    xT = nc.dram_tensor("xT", [128, KC, N], BF16, kind="ExternalInput")
    w_qk = nc.dram_tensor("w_qk", [128, 2 * KC, KC, 128], BF16, kind="ExternalInput")
    w_v = nc.dram_tensor("w_v", [128, KC, D], BF16, kind="ExternalInput")
    w_p = nc.dram_tensor("w_p", [128, KC, D], BF16, kind="ExternalInput")
    bias = nc.dram_tensor("bias", [128, FT], F32, kind="ExternalInput")
    outT = nc.dram_tensor("outT", [D, N], F32, kind="ExternalOutput")

    with ExitStack() as ctx:
        tc = ctx.enter_context(tile.TileContext(nc))
        const = ctx.enter_context(tc.tile_pool(name="const", bufs=1))
        work = ctx.enter_context(tc.tile_pool(name="work", bufs=2))
        # PSUM: 8 banks. psA = 3 rotating [128,1024] f32 slots (6 banks),
        # psB = 1 slot (2 banks, PV accumulator / 4th proj tile).
        psA = ctx.enter_context(tc.tile_pool(name="psA", bufs=3, space="PSUM"))
        psB = ctx.enter_context(tc.tile_pool(name="psB", bufs=1, space="PSUM"))

        xT_sb = const.tile([128, KC, N], BF16)
        wqk_sb = const.tile([128, 2 * KC, KC, 128], BF16)
        wv_sb = const.tile([128, KC, D], BF16)
        wp_sb = const.tile([128, KC, D], BF16)
        bias_sb = const.tile([128, FT], F32)
        qk_sb = const.tile([128, 2 * KC, N], BF16)   # tiles 0-5: qT, 6-11: kT
        v_sb = const.tile([128, NT, H, VW], BF16)
        attn_sb = const.tile([128, KC, N], BF16)     # attn_out^T, normalized

        # --- DMA schedule (priority order per queue; only sync/scalar
        # HWDGE + gpsimd SWDGE can trigger DMAs) ---
        # xT chunks split across sync + scalar so the tensor the whole
        # startup is gated on arrives ~2x faster; first-needed W_qk tiles
        # (q0, k0, then q1, k1) interleaved on scalar.
        for c in (0, 1, 2, 3):
            nc.sync.dma_start(out=xT_sb[:, c, :], in_=xT[:, c, :])
        nc.scalar.dma_start(out=wqk_sb[:, 0], in_=w_qk[:, 0])
        nc.scalar.dma_start(out=wqk_sb[:, KC], in_=w_qk[:, KC])
        for c in (4, 5):
            nc.scalar.dma_start(out=xT_sb[:, c, :], in_=xT[:, c, :])
        for t in (1, KC + 1):
            nc.scalar.dma_start(out=wqk_sb[:, t], in_=w_qk[:, t])
        # Late weights ride the scalar ring's FIFO behind the startup-
        # critical loads, so they can't steal DMA bandwidth from xT/W_v
        # during the ramp. gpsimd (SWDGE) carries only W_v.
        nc.scalar.dma_start(out=wqk_sb[:, 2:KC], in_=w_qk[:, 2:KC])
        nc.scalar.dma_start(out=wqk_sb[:, KC + 2:], in_=w_qk[:, KC + 2:])
        nc.scalar.dma_start(out=wp_sb, in_=w_p[:, :, :])
        nc.scalar.dma_start(out=bias_sb, in_=bias[:, :])
        nc.gpsimd.dma_start(out=wv_sb, in_=w_v[:, :, :])
        # v' scaffold on the (idle-at-startup) DVE: zeros + ones column.
        nc.vector.memset(v_sb, 0.0)
        nc.vector.memset(v_sb[:, :, :, 0:1], 1.0)

        def qkT_ops(t):
            """Closures: 6 accumulation-chunk matmul pairs + the cast copy,
            for interleaving as PE filler inside a head's mt loop."""
            ps_qk = psA.tile([128, N], F32, tag="ps", name="ps_qk")
            ops = []
            for c in range(KC):
                def chunk(c=c, ps_qk=ps_qk):
                    for s in range(2):
                        nc.tensor.matmul(
                            ps_qk[:, 512 * s:512 * (s + 1)],
                            lhsT=wqk_sb[:, t, c, :],
                            rhs=xT_sb[:, c, 512 * s:512 * (s + 1)],
                            start=(c == 0), stop=(c == KC - 1),
                        )
                ops.append(chunk)

            def fin(ps_qk=ps_qk):
                nc.vector.tensor_copy(out=qk_sb[:, t, :], in_=ps_qk)
            ops.append(fin)
            return ops

        def emit_qkT_startup():
            # q-tile 0 and k-tile 0 emitted chunk-major so the PE starts as
            # soon as xT chunk 0 + the first W_qk tile arrive.
            ps0 = psA.tile([128, N], F32, tag="ps", name="ps_qk0")
            ps1 = psA.tile([128, N], F32, tag="ps", name="ps_qk1")
            for c in range(KC):
                for t, ps in ((0, ps0), (KC, ps1)):
                    for s in range(2):
                        nc.tensor.matmul(
                            ps[:, 512 * s:512 * (s + 1)],
                            lhsT=wqk_sb[:, t, c, :],
                            rhs=xT_sb[:, c, 512 * s:512 * (s + 1)],
                            start=(c == 0), stop=(c == KC - 1),
                        )
            nc.vector.tensor_copy(out=qk_sb[:, 0, :], in_=ps0)
            nc.vector.tensor_copy(out=qk_sb[:, KC, :], in_=ps1)

        def emit_v(mt):
            ps_v = psA.tile([128, N], F32, tag="ps", name="ps_v")
            for c in range(KC):
                for lo, sz in ((0, 512), (512, 256)):
                    nc.tensor.matmul(
                        ps_v[:, lo:lo + sz],
                        lhsT=xT_sb[:, c, 128 * mt:128 * (mt + 1)],
                        rhs=wv_sb[:, c, lo:lo + sz],
                        start=(c == 0), stop=(c == KC - 1),
                    )
            nc.vector.tensor_copy(
                out=v_sb[:, mt, :, HD:VW],
                in_=ps_v[:, 0:D].rearrange("p (h e) -> p h e", e=HD),
            )

        ps_o_map = {}
        norm_pending = {}

        def emit_ST_exp(h, mt):
            tq, tk = h // 2, KC + h // 2
            po = (h % 2) * 64
            ps_s = psA.tile([128, N], F32, tag="ps", name="ps_s")
            for s in range(2):
                nc.tensor.matmul(
                    ps_s[:, 512 * s:512 * (s + 1)],
                    lhsT=qk_sb[po:po + 64, tk, 128 * mt:128 * (mt + 1)],
                    rhs=qk_sb[po:po + 64, tq, 512 * s:512 * (s + 1)],
                    start=True, stop=True,
                )
            pt = work.tile([128, N], BF16, tag="pt", name="pt", bufs=8)
            nc.scalar.activation(
                out=pt, in_=ps_s,
                func=mybir.ActivationFunctionType.Exp, scale=SCALE,
            )
            return pt

        def emit_PV(h, mt, pt):
            if mt == 0:
                ps_o_map[h] = psB.tile([128, N], F32, tag="ps", name="ps_o")
            ps_o = ps_o_map[h]
            for s in range(2):
                nc.tensor.matmul(
                    ps_o[:, 512 * s:512 * (s + 1)],
                    lhsT=v_sb[:, mt, h, :],
                    rhs=pt[:, 512 * s:512 * (s + 1)],
                    start=(mt == 0), stop=(mt == NT - 1),
                )

        def emit_norm_a(h):
            # sums on PSUM partition 0 (v' col 0 = ones); v data on
            # partitions 64..127. One bf16 cast moves everything off PSUM
            # so the psB slot frees ~3us earlier (the next head's first PV
            # aliases it); recip/broadcast/mul then work from the copy.
            # reciprocal_approx_fast / partition_broadcast read base
            # partition 0 on HW; DVE can't shift partitions, so even heads
            # hop via SBUF->SBUF DMA.
            ps_o = ps_o_map.pop(h)
            u = work.tile([128, N], F32, tag="u", name="u")
            nc.vector.tensor_copy(out=u, in_=ps_o)
            recip = work.tile([1, N], F32, tag="recip", name="recip")
            nc.vector.reciprocal_approx_fast(out=recip, in_=u[0:1, :])
            rb = work.tile([128, N], F32, tag="rb", name="rb")
            nc.gpsimd.partition_broadcast(rb, recip)
            norm_pending[h] = (u, rb)

        def emit_norm_b(h):
            tq = h // 2
            po = (h % 2) * 64
            u, rb = norm_pending.pop(h)
            if po == 64:
                nc.vector.tensor_mul(
                    out=attn_sb[64:128, tq, :],
                    in0=u[64:128, :], in1=rb[64:128, :],
                )
            else:
                tmp = work.tile([128, N], BF16, tag="tmp", name="tmp")
                nc.vector.tensor_mul(
                    out=tmp[64:128, :], in0=u[64:128, :], in1=rb[64:128, :],
                )
                nc.sync.dma_start(
                    out=attn_sb[0:64, tq, :], in_=tmp[64:128, :],
                )

        # Transposed proj: out^T[ft] = sum_c W_p[c-rows, ft-cols].T @ attn_c
        proj_ps = {}

        def emit_proj_chunks(ft, cs, pool):
            if ft not in proj_ps:
                proj_ps[ft] = pool.tile([128, N], F32, tag="ps", name="ps_p")
            ps_p = proj_ps[ft]
            for c in cs:
                for s in range(2):
                    nc.tensor.matmul(
                        ps_p[:, 512 * s:512 * (s + 1)],
                        lhsT=wp_sb[:, c, 128 * ft:128 * (ft + 1)],
                        rhs=attn_sb[:, c, 512 * s:512 * (s + 1)],
                        start=(c == 0), stop=(c == KC - 1),
                    )

        def emit_proj_fin(ft):
            # halves pipeline: the bias-add + out-DMA of cols 0-511 overlap
            # the second half's matmul drain
            ps_p = proj_ps.pop(ft)
            o_sb = work.tile([128, N], F32, tag="o_sb", name="o_sb", bufs=6)
            dq = nc.sync if ft % 2 == 0 else nc.scalar
            for s in range(2):
                sl = slice(512 * s, 512 * (s + 1))
                nc.vector.tensor_scalar_add(
                    out=o_sb[:, sl], in0=ps_p[:, sl],
                    scalar1=bias_sb[:, ft:ft + 1],
                )
                dq.dma_start(out=outT[128 * ft:128 * (ft + 1), sl], in_=o_sb[:, sl])

        # --- schedule ---
        emit_qkT_startup()
        for mt in range(2):
            emit_v(mt)
        # Filler PE work interleaved inside each head's mt loop, compressed
        # into the first ~6 steps so the parked filler PSUM slot (and its
        # DVE cast) complete before the next head's STs rotate into it.
        head_fillers = {h: [] for h in range(H)}
        head_fillers[0] = [
            (lambda mt=mt: emit_v(mt)) for mt in range(2, NT)
        ] + qkT_ops(1)
        head_fillers[1] = qkT_ops(KC + 1)
        for k in range(1, KC - 1):
            head_fillers[2 * k] = qkT_ops(k + 1)
            head_fillers[2 * k + 1] = qkT_ops(KC + k + 1)

        # Software pipeline: PV lags ST/exp by one mt step.
        LAG = 1
        pending = []
        for h in range(H):
            fl = head_fillers[h]
            fi = 0
            for mt in range(NT):
                pt = emit_ST_exp(h, mt)
                pending.append((h, mt, pt))
                if len(pending) > LAG:
                    ph, pmt, ppt = pending.pop(0)
                    emit_PV(ph, pmt, ppt)
                    if pmt == NT - 1:
                        emit_norm_a(ph)
                    elif pmt == 3 and ph > 0:
                        emit_norm_b(ph - 1)
                while fi < min((mt + 1) * len(fl) // 6, len(fl)):
                    fl[fi]()
                    fi += 1
        # Drain + tail: overlap proj with the last exp/norm chain. The
        # last PVs need the last exps, so a little proj work in front is
        # free PE time.
        emit_proj_chunks(0, range(0, 2), psA)
        (ph, pmt, ppt) = pending.pop(0)
        emit_PV(ph, pmt, ppt)
        emit_norm_a(ph)
        emit_norm_b(ph)
        emit_proj_chunks(0, range(2, 5), psA)
        emit_proj_chunks(1, range(0, 5), psA)
        emit_proj_chunks(2, range(0, 5), psA)
        emit_proj_chunks(3, range(0, 5), psB)   # slot frees after u-cast(11)
        for ft in range(4):
            emit_proj_chunks(ft, range(5, KC), None)
            emit_proj_fin(ft)
        for ft in range(4, KC):
            emit_proj_chunks(ft, range(KC), psA)
            emit_proj_fin(ft)

    nc.compile()
    return nc


def _get_nc():
    if "nc" not in _CACHE:
        _CACHE["nc"] = _build_nc()
    return _CACHE["nc"]


def _make_in_maps(x, W_qkv, W_proj, b_proj):
    bf = ml_dtypes.bfloat16
    x = np.asarray(x, dtype=np.float32)
    W_qkv = np.asarray(W_qkv, dtype=np.float32)
    W_proj = np.asarray(W_proj, dtype=np.float32)
    b_proj = np.asarray(b_proj, dtype=np.float32)
    # Pre-pack DRAM layouts to match SBUF destinations (chunk-major per
    # partition) so each tensor is one dense DMA.
    w_qk = np.ascontiguousarray(
        W_qkv[:, :2 * D].reshape(KC, 128, 2 * KC, 128).transpose(1, 2, 0, 3)
    ).astype(bf)
    w_v = np.ascontiguousarray(
        W_qkv[:, 2 * D:].reshape(KC, 128, D).transpose(1, 0, 2)
    ).astype(bf)
    w_p = np.ascontiguousarray(
        W_proj.reshape(KC, 128, D).transpose(1, 0, 2)
    ).astype(bf)
    bias = np.ascontiguousarray(b_proj.reshape(FT, 128).T)
    xTs = [
        np.ascontiguousarray(
            x[b].T.reshape(KC, 128, N).transpose(1, 0, 2)
        ).astype(bf)
        for b in range(NCORES)
    ]
    return [
        {
            "xT": xTs[b],
            "w_qk": w_qk,
            "w_v": w_v,
            "w_p": w_p,
            "bias": bias,
        }
        for b in range(NCORES)
    ]


def run(x, W_qkv, W_proj, b_proj, trace=False):
    nc = _get_nc()
    in_maps = _make_in_maps(x, W_qkv, W_proj, b_proj)
    res = run_bass_kernel_spmd(nc, in_maps, core_ids=list(range(NCORES)), trace=trace)
    out = np.stack(
        [res.results[b]["outT"].T for b in range(NCORES)], axis=0
    )
    return np.ascontiguousarray(out).astype(np.float32), res


def kernel(x, W_qkv, W_proj, b_proj):
    out, _ = run(x, W_qkv, W_proj, b_proj, trace=False)
    return out
